# revision 17
# baseline (speedup 1.0000x reference)
"""BiDAF attention (nn_BertBidafAttention) on 8 TRN2 NeuronCores.

Math (per batch, reference):
    cp = c @ W.T + b            [CL, H]
    s  = cp @ q.T               [CL, QL]
    s1 = softmax_q(s + qmask_bias)      (row softmax)
    s2 = softmax_c(s + cmask_bias)      (col softmax)
    a  = s1 @ q                 [CL, H]
    bv = (s1 @ s2.T) @ c        [CL, H]
    x  = [c, a, c*a, c*bv]      [CL, 4H]

Restructured for the PE:
    qW[(b q), h'] = sum_h q[(b q), h] W[h, h']    (both batches fused, 75 MF)
    sT[q, c]      = sum_h qWT[h, q] cT[h, c] + cmask_bias[c]  (f32r, 512-wide)
    bv            = s1 @ (s2.T @ c)               (avoids the [CL,CL] product)
The c-mask bias is constant along q for fixed c so it cancels in s1's
softmax; the q-side bias (q.b + qmask_bias) is added per-partition into
sTb and cancels in s2's softmax.  NEGB=-1000 makes exp(masked-max)==0.

Precision split (validated vs the fp32 reference in numpy):
    logit matmuls (qW, sT) run f32r (TF32-ish rounding)  -> ~5e-3 rel
    value matmuls (a, bv, qc) + softmax weights run bf16 -> ~7e-3 rel total
    (bf16 logits would be 7e-2 and fail the 2e-2 gate; f32r values would
     be slower on DVE copies for no accuracy need.)
Softmaxes are kept unnormalized (e1, e2); the 1/sum factors are folded in
as per-partition scales later (e1*r1 before the s1 transpose, r2 on the
qc PSUM->SBUF copy), so no extra normalization passes exist.

Engine split: PE does transposes + matmuls; ScalarE does exp/bias-add and
PSUM->SBUF copies; DVE does reductions, reciprocals and c*bv; GpSimd does
the fp32->bf16 casts of c and c*a.  All outputs are fp32 in SBUF and
written with HWDGE DMAs (inputs on the sync ring, outputs on the scalar
ring so they don't head-of-line-block each other).

Sharding: data-parallel over batch, 2 batches per core, no collectives.
"""

import numpy as np
from contextlib import ExitStack

import concourse.bass as bass
from concourse import bacc
import concourse.mybir as mybir
import concourse.tile as tile
from concourse.masks import make_identity
from concourse.bass_utils import run_bass_kernel_spmd

B, CL, QL, H = 16, 512, 64, 768
NCORES = 8
BPC = B // NCORES  # batches per core
HK = H // 128      # 6 k-tiles over the feature dims
CT = CL // 128     # 4 c-tiles
NH = H // 2        # 384, N per matmul half (one PSUM bank)
NEGB = -1000.0     # additive mask bias; exp(NEGB - max) == 0.0 in fp32

f32 = mybir.dt.float32
f32r = mybir.dt.float32r
bf16 = mybir.dt.bfloat16
i32 = mybir.dt.int32


def _build_nc(precision: int = 1) -> bass.Bass:
    nc = bacc.Bacc()
    cD = nc.declare_dram_parameter("c", [BPC, CL, H], f32, isOutput=False)
    qD = nc.declare_dram_parameter("q", [BPC, QL, H], f32, isOutput=False)
    cmD = nc.declare_dram_parameter("c_mask", [BPC, CL], i32, isOutput=False)
    qmD = nc.declare_dram_parameter("q_mask", [BPC, QL], i32, isOutput=False)
    WD = nc.declare_dram_parameter("W", [H, H], f32, isOutput=False)
    bD = nc.declare_dram_parameter("b", [H], f32, isOutput=False)
    outD = nc.declare_dram_parameter("out", [BPC, CL, 4 * H], f32, isOutput=True)

    ldt = f32r if precision >= 1 else f32  # logit-path matmul dtype

    with tile.TileContext(nc) as tc, ExitStack() as ctx:
        const = ctx.enter_context(tc.tile_pool(name="const", bufs=1))
        wpool = ctx.enter_context(tc.tile_pool(name="wpool", bufs=1))
        cpool = ctx.enter_context(tc.tile_pool(name="cpool", bufs=1))
        small = ctx.enter_context(tc.tile_pool(name="small", bufs=2))
        outp = ctx.enter_context(tc.tile_pool(name="outp", bufs=3))
        ptp = ctx.enter_context(tc.tile_pool(name="ptp", bufs=3, space="PSUM"))
        pacc = ctx.enter_context(tc.tile_pool(name="pacc", bufs=2, space="PSUM"))
        pval = ctx.enter_context(tc.tile_pool(name="pval", bufs=3, space="PSUM"))

        ident = const.tile([128, 128], f32)
        make_identity(nc, ident)
        identb = const.tile([128, 128], bf16)
        nc.vector.tensor_copy(identb, ident)
        ones_f = const.tile([1, 128], f32)
        nc.vector.memset(ones_f, 1.0)
        if ldt == f32:
            ones = ones_f
        else:
            ones = const.tile([1, 128], ldt)
            nc.vector.tensor_copy(ones, ones_f)

        # ---- input DMAs (sync ring; W interleaved between c chunks so the
        # first cT transposes and the k-ordered qW accumulation start early)
        q_both = wpool.tile([128, H], ldt)  # [(b q), h]
        nc.sync.dma_start(out=q_both,
                          in_=qD[:].rearrange("b q h -> (b q) h").bitcast(ldt))
        c_f32s = []
        w_sb = wpool.tile([128, HK, H], ldt)  # [p, k, h']  (W[k*128+p, h'])
        for bi in range(BPC):
            c_f32s.append(cpool.tile([128, CT, H], f32, name=f"c{bi}"))
        nc.sync.dma_start(out=c_f32s[0],
                          in_=cD[0].rearrange("(ct p) h -> p ct h", p=128))
        nc.sync.dma_start(out=w_sb,
                          in_=WD[:].rearrange("(k p) h -> p k h", p=128)
                          .bitcast(ldt))
        nc.sync.dma_start(out=c_f32s[1],
                          in_=cD[1].rearrange("(ct p) h -> p ct h", p=128))
        b_sb = wpool.tile([128, HK], ldt)
        nc.sync.dma_start(out=b_sb,
                          in_=bD[:].rearrange("(k p) -> p k", p=128).bitcast(ldt))

        # bf16 copies of q (SWDGE cast straight from DRAM; per batch so the
        # partition base stays 0)
        q_bf = wpool.tile([64, BPC, H], bf16)
        for bi in range(BPC):
            nc.gpsimd.dma_start(out=q_bf[:, bi, :], in_=qD[bi])

        # ---- mask bias rows (int32 -> fp32 cast during SWDGE DMA) ----
        qmf = small.tile([64, BPC], f32, tag="qmf", bufs=1)
        nc.gpsimd.dma_start(out=qmf, in_=qmD[:].rearrange("b q -> q b"))
        cmf = small.tile([1, BPC, CL], f32, tag="cmf", bufs=1)
        nc.gpsimd.dma_start(out=cmf[:1].rearrange("o b l -> o (b l)"),
                            in_=cmD[:].rearrange("b (o l) -> o (b l)", o=1))
        # bias = (mask - 1) * |NEGB|  ->  0 where mask==1, NEGB where mask==0
        qmbias = small.tile([64, BPC], f32, tag="qmbias", bufs=1)
        nc.scalar.activation(qmbias, qmf, mybir.ActivationFunctionType.Copy,
                             bias=NEGB, scale=-NEGB)
        cbias = small.tile([1, BPC, CL], ldt, tag="cbias", bufs=1)
        nc.scalar.activation(cbias, cmf, mybir.ActivationFunctionType.Copy,
                             bias=NEGB, scale=-NEGB)

        # ---- qT[h, (b q)] via PE transposes of q_both ----
        qT = wpool.tile([128, HK, 128], ldt)
        for g in range(2):  # two groups of 3 k-chunks -> one PSUM bank each
            tp = ptp.tile([128, 3, 128], f32, tag="tp")
            for j in range(3):
                k = g * 3 + j
                nc.tensor.transpose(tp[:, j, :],
                                    q_both.bitcast(f32)[:, k * 128:(k + 1) * 128],
                                    ident)
            nc.vector.tensor_copy(out=qT[:, g * 3:(g + 1) * 3, :], in_=tp)

        # ---- qW[(b q), h'] = q @ W and qb[(b q)] = q . b ----
        qW = wpool.tile([128, H], ldt)
        for hf in range(2):
            ps_qw = pacc.tile([128, 512], f32, tag="acc")
            for k in range(HK):
                nc.tensor.matmul(ps_qw[:, :NH], qT[:, k, :],
                                 w_sb[:, k, hf * NH:(hf + 1) * NH],
                                 start=(k == 0), stop=(k == HK - 1))
            nc.scalar.copy(out=qW[:, hf * NH:(hf + 1) * NH], in_=ps_qw[:, :NH])
        # q-side bias as per-partition column per batch: [64, BPC]
        ps_qb = pacc.tile([64, BPC], f32, tag="acc")
        for bi in range(BPC):
            for k in range(HK):
                # N=1 violates the fp32r moving-dim ISA restriction; these
                # 12 tiny matmuls run as plain fp32 via bitcast views.
                nc.tensor.matmul(ps_qb[:, bi:bi + 1],
                                 qT[:, k, bi * 64:(bi + 1) * 64].bitcast(f32),
                                 b_sb[:, k:k + 1].bitcast(f32),
                                 start=(k == 0), stop=(k == HK - 1))
        qrow = small.tile([64, BPC], f32, tag="qrow", bufs=1)
        nc.vector.tensor_add(qrow, ps_qb, qmbias)

        # ---- qWT[h', (b q)] via PE transposes of qW ----
        qWT = wpool.tile([128, HK, 128], ldt)
        for g in range(2):
            tp = ptp.tile([128, 3, 128], f32, tag="tp")
            for j in range(3):
                k = g * 3 + j
                nc.tensor.transpose(tp[:, j, :],
                                    qW.bitcast(f32)[:, k * 128:(k + 1) * 128],
                                    ident)
            nc.scalar.copy(out=qWT[:, g * 3:(g + 1) * 3, :], in_=tp)

        for bi in range(BPC):
            c_f32 = c_f32s[bi]
            # out[:, 0:H] = c — no compute dependency, write immediately
            nc.scalar.dma_start(
                out=outD[bi].rearrange("(ct p) h4 -> p ct h4", p=128)[:, :, 0:H],
                in_=c_f32)

            # bf16 copy of c for the value-path matmuls / products (GpSimd)
            c_bf = cpool.tile([128, CT, H], bf16, name=f"cb{bi}", tag="cbf")
            for ci in range(CT):
                nc.gpsimd.tensor_copy(out=c_bf[:, ci, :], in_=c_f32[:, ci, :])

            # ---- cT[h, c] via PE transposes, grouped per c-chunk ----
            cT = cpool.tile([128, HK, CL], ldt, name=f"ct{bi}", tag="ctp")
            for ci in range(CT):
                for g in range(2):
                    tp = ptp.tile([128, 3, 128], f32, tag="tp")
                    for j in range(3):
                        k = g * 3 + j
                        nc.tensor.transpose(
                            tp[:, j, :],
                            c_f32[:, ci, k * 128:(k + 1) * 128], ident)
                    eng = nc.vector if (ci + g) % 2 == 0 else nc.scalar
                    if eng is nc.vector:
                        nc.vector.tensor_copy(
                            out=cT[:, g * 3:(g + 1) * 3,
                                   ci * 128:(ci + 1) * 128], in_=tp)
                    else:
                        nc.scalar.copy(
                            out=cT[:, g * 3:(g + 1) * 3,
                                   ci * 128:(ci + 1) * 128], in_=tp)

            # ---- logits sT[q, c] (f32r, 512-wide) + c-mask bias ----
            ps_st = pacc.tile([64, CL], f32, tag="acc")
            for k in range(HK):
                nc.tensor.matmul(ps_st, qWT[:, k, bi * 64:(bi + 1) * 64],
                                 cT[:, k, :], start=(k == 0), stop=False)
            nc.tensor.matmul(ps_st, ones[:1, :QL], cbias[:1, bi],
                             start=False, stop=True)

            # ---- column softmax s2 (over c = free axis), unnormalized ----
            nmax2 = small.tile([64, 1], f32, tag="nmax2")
            nc.vector.reduce_max(nmax2, ps_st, axis=mybir.AxisListType.X,
                                 negate=True)
            e2T = small.tile([64, CL], bf16, tag="e2T")
            sum2 = small.tile([64, 1], f32, tag="sum2")
            nc.scalar.activation(e2T, ps_st, mybir.ActivationFunctionType.Exp,
                                 bias=nmax2, scale=1.0, accum_out=sum2)
            r2 = small.tile([64, 1], f32, tag="r2")
            nc.vector.reciprocal(r2, sum2)
            # e2[c, q] chunks for the qc matmul (transpose back)
            tpe = ptp.tile([128, CT, QL], bf16, tag="tp")
            for ci in range(CT):
                nc.tensor.transpose(tpe[:, ci, :],
                                    e2T[:, ci * 128:(ci + 1) * 128],
                                    identb[:64, :64])
            e2s = small.tile([128, CT, QL], bf16, tag="e2s")
            nc.vector.tensor_copy(out=e2s, in_=tpe)

            # biased logits (+ q-side bias, per partition) for the s1 path
            sTb = small.tile([64, CL], f32, tag="sTb")
            nc.scalar.add(sTb, ps_st, qrow[:, bi:bi + 1])

            # ---- row softmax s1: transpose to [c, q], softmax over free q,
            # scale by 1/sum, transpose back to s1T[q, c] (bf16) ----
            tps = ptp.tile([128, CT, QL], f32, tag="tp")
            for ci in range(CT):
                nc.tensor.transpose(tps[:, ci, :],
                                    sTb[:, ci * 128:(ci + 1) * 128],
                                    ident[:64, :64])
            e1 = small.tile([128, CT, QL], bf16, tag="e1")
            sum1 = small.tile([128, CT], f32, tag="sum1")
            for ci in range(CT):
                nmax1 = small.tile([128, 1], f32, tag="nmax1")
                nc.vector.reduce_max(nmax1, tps[:, ci, :],
                                     axis=mybir.AxisListType.X, negate=True)
                nc.scalar.activation(e1[:, ci, :], tps[:, ci, :],
                                     mybir.ActivationFunctionType.Exp,
                                     bias=nmax1, scale=1.0,
                                     accum_out=sum1[:, ci:ci + 1])
            r1 = small.tile([128, CT], f32, tag="r1")
            nc.vector.reciprocal(r1, sum1)
            s1 = small.tile([128, CT, QL], bf16, tag="s1")
            for ci in range(CT):
                nc.scalar.mul(s1[:, ci, :], e1[:, ci, :], r1[:, ci:ci + 1])
            tpt = ptp.tile([64, CL], bf16, tag="tp")
            for ci in range(CT):
                nc.tensor.transpose(tpt[:, ci * 128:(ci + 1) * 128],
                                    s1[:, ci, :], identb)
            s1T = small.tile([64, CL], bf16, tag="s1T")
            nc.vector.tensor_copy(out=s1T, in_=tpt)

            # ---- qc[q, h] = s2.T @ c  (bf16, scale r2 on copy-out) ----
            qc_bf = small.tile([64, H], bf16, tag="qc")
            for hf in range(2):
                ps_qc = pacc.tile([64, 512], f32, tag="acc")
                for ci in range(CT):
                    nc.tensor.matmul(ps_qc[:, :NH], e2s[:, ci, :],
                                     c_bf[:, ci, hf * NH:(hf + 1) * NH],
                                     start=(ci == 0), stop=(ci == CT - 1))
                nc.scalar.mul(qc_bf[:, hf * NH:(hf + 1) * NH],
                              ps_qc[:, :NH], r2)

            # ---- a = s1 @ q ; bv = s1 @ qc ; products; outputs ----
            a_f32 = outp.tile([128, CT, H], f32, tag="af", bufs=2)
            ca_f32 = outp.tile([128, CT, H], f32, tag="caf", bufs=2)
            cbv_f32 = outp.tile([128, CT, H], f32, tag="cbvf", bufs=2)
            for ci in range(CT):
                for hf in range(2):
                    cols = slice(hf * NH, (hf + 1) * NH)
                    ps_a = pval.tile([128, 512], f32, tag="val")
                    nc.tensor.matmul(ps_a[:, :NH],
                                     s1T[:, ci * 128:(ci + 1) * 128],
                                     q_bf[:, bi, cols], start=True, stop=True)
                    nc.scalar.copy(out=a_f32[:, ci, cols], in_=ps_a[:, :NH])
                    ps_bv = pval.tile([128, 512], f32, tag="val")
                    nc.tensor.matmul(ps_bv[:, :NH],
                                     s1T[:, ci * 128:(ci + 1) * 128],
                                     qc_bf[:, cols], start=True, stop=True)
                    nc.vector.tensor_mul(cbv_f32[:, ci, cols],
                                         c_f32[:, ci, cols], ps_bv[:, :NH])
                nc.vector.tensor_mul(ca_f32[:, ci, :], c_f32[:, ci, :],
                                     a_f32[:, ci, :])
            out_r = outD[bi].rearrange("(ct p) h4 -> p ct h4", p=128)
            nc.scalar.dma_start(out=out_r[:, :, H:2 * H], in_=a_f32)
            nc.scalar.dma_start(out=out_r[:, :, 2 * H:3 * H], in_=ca_f32)
            nc.scalar.dma_start(out=out_r[:, :, 3 * H:4 * H], in_=cbv_f32)

    nc.finalize()
    return nc


_NC_CACHE: dict = {}


def _get_nc(precision: int = 1) -> bass.Bass:
    if precision not in _NC_CACHE:
        _NC_CACHE[precision] = _build_nc(precision)
    return _NC_CACHE[precision]


def kernel(c, q, c_mask, q_mask, W, b, _trace=False, _precision=1, _tmpdir=None):
    nc = _get_nc(_precision)
    in_maps = []
    for i in range(NCORES):
        sl = slice(i * BPC, (i + 1) * BPC)
        in_maps.append({
            "c": np.ascontiguousarray(np.asarray(c)[sl], dtype=np.float32),
            "q": np.ascontiguousarray(np.asarray(q)[sl], dtype=np.float32),
            "c_mask": np.ascontiguousarray(np.asarray(c_mask)[sl], dtype=np.int32),
            "q_mask": np.ascontiguousarray(np.asarray(q_mask)[sl], dtype=np.int32),
            "W": np.ascontiguousarray(np.asarray(W), dtype=np.float32),
            "b": np.ascontiguousarray(np.asarray(b), dtype=np.float32),
        })
    res = run_bass_kernel_spmd(nc, in_maps, core_ids=list(range(NCORES)),
                               trace=_trace, tmpdir=_tmpdir)
    out = np.concatenate([res.results[i]["out"] for i in range(NCORES)], axis=0)
    if _trace:
        return out, res
    return out


# revision 20
# speedup vs baseline: 1.3960x; 1.3960x over previous
"""BiDAF attention (nn_BertBidafAttention) on 8 TRN2 NeuronCores.

Math (per batch, reference):
    cp = c @ W.T + b            [CL, H]
    s  = cp @ q.T               [CL, QL]
    s1 = softmax_q(s + qmask_bias)      (row softmax)
    s2 = softmax_c(s + cmask_bias)      (col softmax)
    a  = s1 @ q                 [CL, H]
    bv = (s1 @ s2.T) @ c        [CL, H]
    x  = [c, a, c*a, c*bv]      [CL, 4H]

Restructured for the PE:
    qW[(b q), h'] = sum_h q[(b q), h] W[h, h']    (both batches fused, 75 MF)
    sT[q, c]      = sum_h qWT[h, q] cT[h, c] + cmask_bias[c]  (f32r, 512-wide)
    bv            = s1 @ (s2.T @ c)               (avoids the [CL,CL] product)
The c-mask bias is constant along q for fixed c so it cancels in s1's
softmax; the q-side bias (q.b + qmask_bias) is added per-partition into
sTb and cancels in s2's softmax.  NEGB=-1000 makes exp(masked-max)==0.

Precision split (validated vs the fp32 reference in numpy):
    logit matmuls (qW, sT) run f32r (TF32-ish rounding)  -> ~5e-3 rel
    value matmuls (a, bv, qc) + softmax weights run bf16 -> ~7e-3 rel total
Softmaxes are kept unnormalized (e1, e2); the 1/sum factors are folded in
as per-partition scales (e1*r1 before the s1 transpose, r2 on the qc
PSUM->SBUF copy).

Scheduling notes (from per-instruction NTFF traces):
  - every HWDGE dma_start costs ~0.8us of *sequencer* time, so there is
    one DMA per batch/quarter and they all live on the sync ring (which
    has no compute), ordered by expected readiness; tiny/irregular loads
    (b, masks, q_bf) go through SWDGE on gpsimd instead.
  - GpSimd tensor ops are ~4x slower than DVE, so the fp32->bf16 casts of
    c are split between DVE and ACT and gpsimd only does SWDGE loads.
  - ps_st gets its own PSUM tag so batch 1's logit matmuls don't wait for
    batch 0's softmax to release the accumulator bank.

Sharding: data-parallel over batch, 2 batches per core, no collectives.
"""

import numpy as np
from contextlib import ExitStack

import concourse.bass as bass
from concourse import bacc
import concourse.mybir as mybir
import concourse.tile as tile
from concourse.masks import make_identity
from concourse.bass_utils import run_bass_kernel_spmd

B, CL, QL, H = 16, 512, 64, 768
NCORES = 8
BPC = B // NCORES  # batches per core
HK = H // 128      # 6 k-tiles over the feature dims
CT = CL // 128     # 4 c-tiles
NH = H // 2        # 384, N per matmul half (one PSUM bank)
NEGB = -1000.0     # additive mask bias; exp(NEGB - max) == 0.0 in fp32

f32 = mybir.dt.float32
f32r = mybir.dt.float32r
bf16 = mybir.dt.bfloat16
i32 = mybir.dt.int32


def _build_nc(precision: int = 1) -> bass.Bass:
    nc = bacc.Bacc()
    cD = nc.declare_dram_parameter("c", [BPC, CL, H], f32, isOutput=False)
    qD = nc.declare_dram_parameter("q", [BPC, QL, H], f32, isOutput=False)
    cmD = nc.declare_dram_parameter("c_mask", [BPC, CL], i32, isOutput=False)
    qmD = nc.declare_dram_parameter("q_mask", [BPC, QL], i32, isOutput=False)
    WD = nc.declare_dram_parameter("W", [H, H], f32, isOutput=False)
    bD = nc.declare_dram_parameter("b", [H], f32, isOutput=False)
    outD = nc.declare_dram_parameter("out", [BPC, CL, 4 * H], f32, isOutput=True)

    ldt = f32r if precision >= 1 else f32  # logit-path matmul dtype

    with tile.TileContext(nc) as tc, ExitStack() as ctx:
        const = ctx.enter_context(tc.tile_pool(name="const", bufs=1))
        wpool = ctx.enter_context(tc.tile_pool(name="wpool", bufs=1))
        cpool = ctx.enter_context(tc.tile_pool(name="cpool", bufs=1))
        small = ctx.enter_context(tc.tile_pool(name="small", bufs=2))
        outp = ctx.enter_context(tc.tile_pool(name="outp", bufs=2))
        ptp = ctx.enter_context(tc.tile_pool(name="ptp", bufs=3, space="PSUM"))
        pst = ctx.enter_context(tc.tile_pool(name="pst", bufs=2, space="PSUM"))
        pval = ctx.enter_context(tc.tile_pool(name="pval", bufs=3, space="PSUM"))

        ident = const.tile([128, 128], f32)
        make_identity(nc, ident)
        identb = const.tile([128, 128], bf16)
        nc.vector.tensor_copy(identb, ident)
        ones_f = const.tile([1, 128], f32)
        nc.vector.memset(ones_f, 1.0)
        if ldt == f32:
            ones = ones_f
        else:
            ones = const.tile([1, 128], ldt)
            nc.vector.tensor_copy(ones, ones_f)

        # ---- input DMAs: one per big tensor, all on the sync ring ----
        q_both = wpool.tile([128, H], ldt)  # [(b q), h]
        nc.sync.dma_start(out=q_both,
                          in_=qD[:].rearrange("b q h -> (b q) h").bitcast(ldt))
        c_f32s = []
        for bi in range(BPC):
            c_f32s.append(cpool.tile([128, CT, H], f32, name=f"c{bi}"))
        w_sb = wpool.tile([128, HK, H], ldt)  # [p, k, h']  (W[k*128+p, h'])
        nc.sync.dma_start(out=c_f32s[0],
                          in_=cD[0].rearrange("(ct p) h -> p ct h", p=128))
        nc.sync.dma_start(out=w_sb,
                          in_=WD[:].rearrange("(k p) h -> p k h", p=128)
                          .bitcast(ldt))
        nc.sync.dma_start(out=c_f32s[1],
                          in_=cD[1].rearrange("(ct p) h -> p ct h", p=128))

        # small/irregular loads via SWDGE (gpsimd), casting where needed
        b_sb = wpool.tile([128, HK], f32)
        nc.gpsimd.dma_start(out=b_sb,
                            in_=bD[:].rearrange("(k p) -> p k", p=128))
        q_bf = wpool.tile([64, BPC, H], bf16)
        for bi in range(BPC):
            nc.gpsimd.dma_start(out=q_bf[:, bi, :], in_=qD[bi])
        qmf = small.tile([64, BPC], f32, tag="qmf", bufs=1)
        nc.gpsimd.dma_start(out=qmf, in_=qmD[:].rearrange("b q -> q b"))
        cmf = small.tile([1, BPC, CL], f32, tag="cmf", bufs=1)
        nc.gpsimd.dma_start(out=cmf[:1].rearrange("o b l -> o (b l)"),
                            in_=cmD[:].rearrange("b (o l) -> o (b l)", o=1))
        # bias = (mask - 1) * |NEGB|  ->  0 where mask==1, NEGB where mask==0
        qmbias = small.tile([64, BPC], f32, tag="qmbias", bufs=1)
        nc.scalar.activation(qmbias, qmf, mybir.ActivationFunctionType.Copy,
                             bias=NEGB, scale=-NEGB)
        cbias = small.tile([1, BPC, CL], ldt, tag="cbias", bufs=1)
        nc.scalar.activation(cbias, cmf, mybir.ActivationFunctionType.Copy,
                             bias=NEGB, scale=-NEGB)

        # ---- qT[h, (b q)] via PE transposes of q_both ----
        qT = wpool.tile([128, HK, 128], ldt)
        for g in range(2):  # two groups of 3 k-chunks -> one PSUM bank each
            tp = ptp.tile([128, 3, 128], f32, tag="tp")
            for j in range(3):
                k = g * 3 + j
                nc.tensor.transpose(tp[:, j, :],
                                    q_both.bitcast(f32)[:, k * 128:(k + 1) * 128],
                                    ident)
            nc.vector.tensor_copy(out=qT[:, g * 3:(g + 1) * 3, :], in_=tp)

        # ---- cT[h, c] via PE transposes, grouped per c-chunk ----
        cTs = []
        c_bfs = []
        for bi in range(BPC):
            c_f32 = c_f32s[bi]
            cT = cpool.tile([128, HK, CL], ldt, name=f"ct{bi}")
            for ci in range(CT):
                for g in range(2):
                    tp = ptp.tile([128, 3, 128], f32, tag="tp")
                    for j in range(3):
                        k = g * 3 + j
                        nc.tensor.transpose(
                            tp[:, j, :],
                            c_f32[:, ci, k * 128:(k + 1) * 128], ident)
                    if (ci + g) % 2 == 0:
                        nc.vector.tensor_copy(
                            out=cT[:, g * 3:(g + 1) * 3,
                                   ci * 128:(ci + 1) * 128], in_=tp)
                    else:
                        nc.scalar.copy(
                            out=cT[:, g * 3:(g + 1) * 3,
                                   ci * 128:(ci + 1) * 128], in_=tp)
            cTs.append(cT)
            # bf16 copy of c for the value-path matmuls / products
            c_bf = cpool.tile([128, CT, H], bf16, name=f"cb{bi}")
            for ci in range(CT):
                if ci % 2 == 0:
                    nc.vector.tensor_copy(out=c_bf[:, ci, :],
                                          in_=c_f32[:, ci, :])
                else:
                    nc.scalar.copy(out=c_bf[:, ci, :], in_=c_f32[:, ci, :])
            c_bfs.append(c_bf)

        # ---- qW[(b q), h'] = q @ W and qb[(b q)] = q . b ----
        qW = wpool.tile([128, H], ldt)
        for hf in range(2):
            ps_qw = pst.tile([128, 512], f32, tag="st")
            for k in range(HK):
                nc.tensor.matmul(ps_qw[:, :NH], qT[:, k, :],
                                 w_sb[:, k, hf * NH:(hf + 1) * NH],
                                 start=(k == 0), stop=(k == HK - 1))
            nc.scalar.copy(out=qW[:, hf * NH:(hf + 1) * NH], in_=ps_qw[:, :NH])
        ps_qb = pst.tile([64, BPC], f32, tag="st")
        for bi in range(BPC):
            for k in range(HK):
                # N=1 violates the fp32r moving-dim ISA restriction; these
                # 12 tiny matmuls run as plain fp32 via bitcast views.
                nc.tensor.matmul(ps_qb[:, bi:bi + 1],
                                 qT[:, k, bi * 64:(bi + 1) * 64].bitcast(f32),
                                 b_sb[:, k:k + 1],
                                 start=(k == 0), stop=(k == HK - 1))
        qrow = small.tile([64, BPC], f32, tag="qrow", bufs=1)
        nc.vector.tensor_add(qrow, ps_qb, qmbias)

        # ---- qWT[h', (b q)] via PE transposes of qW ----
        qWT = wpool.tile([128, HK, 128], ldt)
        for g in range(2):
            tp = ptp.tile([128, 3, 128], f32, tag="tp")
            for j in range(3):
                k = g * 3 + j
                nc.tensor.transpose(tp[:, j, :],
                                    qW.bitcast(f32)[:, k * 128:(k + 1) * 128],
                                    ident)
            nc.scalar.copy(out=qWT[:, g * 3:(g + 1) * 3, :], in_=tp)

        # ---- logits sT[q, c] (f32r, 512-wide) + c-mask bias, per batch ----
        ps_sts = []
        for bi in range(BPC):
            ps_st = pst.tile([64, CL], f32, tag="st")
            for k in range(HK):
                nc.tensor.matmul(ps_st, qWT[:, k, bi * 64:(bi + 1) * 64],
                                 cTs[bi][:, k, :], start=(k == 0), stop=False)
            nc.tensor.matmul(ps_st, ones[:1, :QL], cbias[:1, bi],
                             start=False, stop=True)
            ps_sts.append(ps_st)

        # ---- softmaxes + value matmuls + products + output DMAs ----
        out0_done = False
        for bi in range(BPC):
            c_f32, c_bf, cT, ps_st = (c_f32s[bi], c_bfs[bi], cTs[bi],
                                      ps_sts[bi])
            out_r = outD[bi].rearrange("(ct p) h4 -> p ct h4", p=128)
            # out[:, 0:H] = c — no compute dependency, write early
            nc.sync.dma_start(out=out_r[:, :, 0:H], in_=c_f32)

            # column softmax s2 (over c = free axis), unnormalized
            nmax2 = small.tile([64, 1], f32, tag="nmax2")
            nc.vector.reduce_max(nmax2, ps_st, axis=mybir.AxisListType.X,
                                 negate=True)
            e2T = small.tile([64, CL], bf16, tag="e2T")
            sum2 = small.tile([64, 1], f32, tag="sum2")
            nc.scalar.activation(e2T, ps_st, mybir.ActivationFunctionType.Exp,
                                 bias=nmax2, scale=1.0, accum_out=sum2)
            r2 = small.tile([64, 1], f32, tag="r2")
            nc.vector.reciprocal(r2, sum2)
            # e2[c, q] chunks for the qc matmul (transpose back)
            tpe = ptp.tile([128, CT, QL], bf16, tag="tp")
            for ci in range(CT):
                nc.tensor.transpose(tpe[:, ci, :],
                                    e2T[:, ci * 128:(ci + 1) * 128],
                                    identb[:64, :64])
            e2s = small.tile([128, CT, QL], bf16, tag="e2s")
            nc.vector.tensor_copy(out=e2s, in_=tpe)

            # biased logits (+ per-partition q-side bias) for the s1 path
            sTb = small.tile([64, CL], f32, tag="sTb")
            nc.scalar.add(sTb, ps_st, qrow[:, bi:bi + 1])

            # row softmax s1 in the [c, q] layout, unnormalized + scaled
            tps = ptp.tile([128, CT, QL], f32, tag="tp")
            for ci in range(CT):
                nc.tensor.transpose(tps[:, ci, :],
                                    sTb[:, ci * 128:(ci + 1) * 128],
                                    ident[:64, :64])
            e1 = small.tile([128, CT, QL], bf16, tag="e1")
            sum1 = small.tile([128, CT], f32, tag="sum1")
            for ci in range(CT):
                nmax1 = small.tile([128, 1], f32, tag="nmax1")
                nc.vector.reduce_max(nmax1, tps[:, ci, :],
                                     axis=mybir.AxisListType.X, negate=True)
                nc.scalar.activation(e1[:, ci, :], tps[:, ci, :],
                                     mybir.ActivationFunctionType.Exp,
                                     bias=nmax1, scale=1.0,
                                     accum_out=sum1[:, ci:ci + 1])
            r1 = small.tile([128, CT], f32, tag="r1")
            nc.vector.reciprocal(r1, sum1)
            s1 = small.tile([128, CT, QL], bf16, tag="s1")
            for ci in range(CT):
                nc.scalar.mul(s1[:, ci, :], e1[:, ci, :], r1[:, ci:ci + 1])
            tpt = ptp.tile([64, CL], bf16, tag="tp")
            for ci in range(CT):
                nc.tensor.transpose(tpt[:, ci * 128:(ci + 1) * 128],
                                    s1[:, ci, :], identb)
            s1T = small.tile([64, CL], bf16, tag="s1T")
            nc.vector.tensor_copy(out=s1T, in_=tpt)

            # qc[q, h] = s2.T @ c  (bf16, scale r2 on copy-out)
            qc_bf = small.tile([64, H], bf16, tag="qc")
            for hf in range(2):
                ps_qc = pval.tile([64, 512], f32, tag="val")
                for ci in range(CT):
                    nc.tensor.matmul(ps_qc[:, :NH], e2s[:, ci, :],
                                     c_bf[:, ci, hf * NH:(hf + 1) * NH],
                                     start=(ci == 0), stop=(ci == CT - 1))
                nc.scalar.mul(qc_bf[:, hf * NH:(hf + 1) * NH],
                              ps_qc[:, :NH], r2)

            # a = s1 @ q ; bv = s1 @ qc ; c*a ; c*bv
            a_f32 = outp.tile([128, CT, H], f32, tag="af")
            ca_f32 = outp.tile([128, CT, H], f32, tag="caf")
            cbv_f32 = outp.tile([128, CT, H], f32, tag="cbvf")
            for ci in range(CT):
                for hf in range(2):
                    cols = slice(hf * NH, (hf + 1) * NH)
                    ps_a = pval.tile([128, 512], f32, tag="val")
                    nc.tensor.matmul(ps_a[:, :NH],
                                     s1T[:, ci * 128:(ci + 1) * 128],
                                     q_bf[:, bi, cols], start=True, stop=True)
                    nc.scalar.copy(out=a_f32[:, ci, cols], in_=ps_a[:, :NH])
                    ps_bv = pval.tile([128, 512], f32, tag="val")
                    nc.tensor.matmul(ps_bv[:, :NH],
                                     s1T[:, ci * 128:(ci + 1) * 128],
                                     qc_bf[:, cols], start=True, stop=True)
                    nc.vector.tensor_mul(cbv_f32[:, ci, cols],
                                         c_f32[:, ci, cols], ps_bv[:, :NH])
                nc.vector.tensor_mul(ca_f32[:, ci, :], c_f32[:, ci, :],
                                     a_f32[:, ci, :])
            nc.sync.dma_start(out=out_r[:, :, H:2 * H], in_=a_f32)
            nc.sync.dma_start(out=out_r[:, :, 2 * H:3 * H], in_=ca_f32)
            nc.sync.dma_start(out=out_r[:, :, 3 * H:4 * H], in_=cbv_f32)

    nc.finalize()
    return nc


_NC_CACHE: dict = {}


def _get_nc(precision: int = 1) -> bass.Bass:
    if precision not in _NC_CACHE:
        _NC_CACHE[precision] = _build_nc(precision)
    return _NC_CACHE[precision]


def kernel(c, q, c_mask, q_mask, W, b, _trace=False, _precision=1, _tmpdir=None):
    nc = _get_nc(_precision)
    in_maps = []
    for i in range(NCORES):
        sl = slice(i * BPC, (i + 1) * BPC)
        in_maps.append({
            "c": np.ascontiguousarray(np.asarray(c)[sl], dtype=np.float32),
            "q": np.ascontiguousarray(np.asarray(q)[sl], dtype=np.float32),
            "c_mask": np.ascontiguousarray(np.asarray(c_mask)[sl], dtype=np.int32),
            "q_mask": np.ascontiguousarray(np.asarray(q_mask)[sl], dtype=np.int32),
            "W": np.ascontiguousarray(np.asarray(W), dtype=np.float32),
            "b": np.ascontiguousarray(np.asarray(b), dtype=np.float32),
        })
    res = run_bass_kernel_spmd(nc, in_maps, core_ids=list(range(NCORES)),
                               trace=_trace, tmpdir=_tmpdir)
    out = np.concatenate([res.results[i]["out"] for i in range(NCORES)], axis=0)
    if _trace:
        return out, res
    return out


# revision 21
# speedup vs baseline: 1.4175x; 1.0154x over previous
"""BiDAF attention (nn_BertBidafAttention) on 8 TRN2 NeuronCores.

Math (per batch, reference):
    cp = c @ W.T + b            [CL, H]
    s  = cp @ q.T               [CL, QL]
    s1 = softmax_q(s + qmask_bias)      (row softmax)
    s2 = softmax_c(s + cmask_bias)      (col softmax)
    a  = s1 @ q                 [CL, H]
    bv = (s1 @ s2.T) @ c        [CL, H]
    x  = [c, a, c*a, c*bv]      [CL, 4H]

Restructured for the PE:
    qW[(b q), h'] = sum_h q[(b q), h] W[h, h']    (both batches fused, 75 MF)
    sT[q, c]      = sum_h qWT[h, q] cT[h, c] + cmask_bias[c]  (f32r, 512-wide)
    bv            = s1 @ (s2.T @ c)               (avoids the [CL,CL] product)
The c-mask bias is constant along q for fixed c so it cancels in s1's
softmax; the q-side bias (q.b + qmask_bias) is added per-partition into
sTb and cancels in s2's softmax.  NEGB=-1000 makes exp(masked-max)==0.

Precision split (validated vs the fp32 reference in numpy):
    logit matmuls (qW, sT) and the qc matmul run f32r (TF32-ish rounding);
    the a/bv matmuls run bf16 on the softmax weights.  (bf16 logits would
    be 7e-2 rel err and fail the 2e-2 gate.)
Both softmaxes stay unnormalized: e2's 1/sum rides the qc copy-out, e1's
1/sum rides the a copy-out and the c*bv product (scalar_tensor_tensor),
so no normalization passes exist at all.

Scheduling notes (from per-instruction NTFF traces):
  - every HWDGE dma_start costs ~0.8us of *sequencer* time, so DMAs are
    batched (1 per input tensor, per-ci-pair for outputs) and all live on
    the sync ring (no compute), ordered by expected readiness; tiny or
    casting loads (b, masks, q_bf) go through SWDGE on gpsimd.
  - GpSimd tensor ops are ~4x slower than DVE -> gpsimd only does SWDGE.
  - c is held in SBUF as f32r (same bits as fp32): the qc matmul streams
    it directly, fp32 consumers use bitcast views, so no bf16 copy of c.
  - ps_st has its own PSUM tag so batch 1's logit matmuls don't wait on
    batch 0's softmax to release the bank.

Sharding: data-parallel over batch, 2 batches per core, no collectives.
"""

import numpy as np
from contextlib import ExitStack

import concourse.bass as bass
from concourse import bacc
import concourse.mybir as mybir
import concourse.tile as tile
from concourse.masks import make_identity
from concourse.bass_utils import run_bass_kernel_spmd

B, CL, QL, H = 16, 512, 64, 768
NCORES = 8
BPC = B // NCORES  # batches per core
HK = H // 128      # 6 k-tiles over the feature dims
CT = CL // 128     # 4 c-tiles
NH = H // 2        # 384, N per matmul half (one PSUM bank)
NEGB = -1000.0     # additive mask bias; exp(NEGB - max) == 0.0 in fp32

f32 = mybir.dt.float32
f32r = mybir.dt.float32r
bf16 = mybir.dt.bfloat16
i32 = mybir.dt.int32
MULT = mybir.AluOpType.mult


def _build_nc(precision: int = 1) -> bass.Bass:
    nc = bacc.Bacc()
    cD = nc.declare_dram_parameter("c", [BPC, CL, H], f32, isOutput=False)
    qD = nc.declare_dram_parameter("q", [BPC, QL, H], f32, isOutput=False)
    cmD = nc.declare_dram_parameter("c_mask", [BPC, CL], i32, isOutput=False)
    qmD = nc.declare_dram_parameter("q_mask", [BPC, QL], i32, isOutput=False)
    WD = nc.declare_dram_parameter("W", [H, H], f32, isOutput=False)
    bD = nc.declare_dram_parameter("b", [H], f32, isOutput=False)
    outD = nc.declare_dram_parameter("out", [BPC, CL, 4 * H], f32, isOutput=True)

    ldt = f32r if precision >= 1 else f32  # logit-path matmul dtype

    with tile.TileContext(nc) as tc, ExitStack() as ctx:
        const = ctx.enter_context(tc.tile_pool(name="const", bufs=1))
        wpool = ctx.enter_context(tc.tile_pool(name="wpool", bufs=1))
        cpool = ctx.enter_context(tc.tile_pool(name="cpool", bufs=1))
        small = ctx.enter_context(tc.tile_pool(name="small", bufs=2))
        outp = ctx.enter_context(tc.tile_pool(name="outp", bufs=2))
        ptp = ctx.enter_context(tc.tile_pool(name="ptp", bufs=3, space="PSUM"))
        pst = ctx.enter_context(tc.tile_pool(name="pst", bufs=2, space="PSUM"))
        pval = ctx.enter_context(tc.tile_pool(name="pval", bufs=3, space="PSUM"))

        ident = const.tile([128, 128], f32)
        make_identity(nc, ident)
        identb = const.tile([128, 128], bf16)
        nc.vector.tensor_copy(identb, ident)
        ones_f = const.tile([1, 128], f32)
        nc.vector.memset(ones_f, 1.0)
        if ldt == f32:
            ident_r = ident
            ones = ones_f
        else:
            ident_r = const.tile([128, 128], ldt)
            nc.vector.tensor_copy(ident_r, ident)
            ones = const.tile([1, 128], ldt)
            nc.vector.tensor_copy(ones, ones_f)

        # ---- input DMAs: one per big tensor, all on the sync ring ----
        q_both = wpool.tile([128, H], ldt)  # [(b q), h]
        nc.sync.dma_start(out=q_both,
                          in_=qD[:].rearrange("b q h -> (b q) h").bitcast(ldt))
        c_rs, c_fs = [], []
        for bi in range(BPC):
            c_r = cpool.tile([128, CT, H], ldt, name=f"c{bi}")
            c_rs.append(c_r)
            c_fs.append(c_r.bitcast(f32))
        w_sb = wpool.tile([128, HK, H], ldt)  # [p, k, h']  (W[k*128+p, h'])
        nc.sync.dma_start(out=c_rs[0],
                          in_=cD[0].rearrange("(ct p) h -> p ct h", p=128)
                          .bitcast(ldt))
        nc.sync.dma_start(out=w_sb,
                          in_=WD[:].rearrange("(k p) h -> p k h", p=128)
                          .bitcast(ldt))
        nc.sync.dma_start(out=c_rs[1],
                          in_=cD[1].rearrange("(ct p) h -> p ct h", p=128)
                          .bitcast(ldt))

        # small/irregular loads via SWDGE (gpsimd), casting where needed
        b_sb = wpool.tile([128, HK], f32)
        nc.gpsimd.dma_start(out=b_sb,
                            in_=bD[:].rearrange("(k p) -> p k", p=128))
        q_bf = wpool.tile([64, BPC, H], bf16)
        for bi in range(BPC):
            nc.gpsimd.dma_start(out=q_bf[:, bi, :], in_=qD[bi])
        qmf = small.tile([64, BPC], f32, tag="qmf", bufs=1)
        nc.gpsimd.dma_start(out=qmf, in_=qmD[:].rearrange("b q -> q b"))
        cmf = small.tile([1, BPC, CL], f32, tag="cmf", bufs=1)
        nc.gpsimd.dma_start(out=cmf[:1].rearrange("o b l -> o (b l)"),
                            in_=cmD[:].rearrange("b (o l) -> o (b l)", o=1))
        # bias = (mask - 1) * |NEGB|  ->  0 where mask==1, NEGB where mask==0
        qmbias = small.tile([64, BPC], f32, tag="qmbias", bufs=1)
        nc.scalar.activation(qmbias, qmf, mybir.ActivationFunctionType.Copy,
                             bias=NEGB, scale=-NEGB)
        cbias = small.tile([1, BPC, CL], ldt, tag="cbias", bufs=1)
        nc.scalar.activation(cbias, cmf, mybir.ActivationFunctionType.Copy,
                             bias=NEGB, scale=-NEGB)

        # ---- qT[h, (b q)] via PE transposes of q_both ----
        qT = wpool.tile([128, HK, 128], ldt)
        for g in range(2):  # two groups of 3 k-chunks -> one PSUM bank each
            tp = ptp.tile([128, 3, 128], ldt, tag="tp")
            for j in range(3):
                k = g * 3 + j
                nc.tensor.transpose(tp[:, j, :],
                                    q_both[:, k * 128:(k + 1) * 128], ident_r)
            nc.vector.tensor_copy(out=qT[:, g * 3:(g + 1) * 3, :], in_=tp)

        # ---- cT[h, c] via PE transposes, grouped per c-chunk ----
        cTs = []
        for bi in range(BPC):
            c_r = c_rs[bi]
            cT = cpool.tile([128, HK, CL], ldt, name=f"ct{bi}")
            for ci in range(CT):
                for g in range(2):
                    tp = ptp.tile([128, 3, 128], ldt, tag="tp")
                    for j in range(3):
                        k = g * 3 + j
                        nc.tensor.transpose(
                            tp[:, j, :],
                            c_r[:, ci, k * 128:(k + 1) * 128], ident_r)
                    dst = cT[:, g * 3:(g + 1) * 3, ci * 128:(ci + 1) * 128]
                    if (ci + g) % 2 == 0:
                        nc.vector.tensor_copy(out=dst, in_=tp)
                    else:
                        nc.scalar.copy(out=dst, in_=tp)
            cTs.append(cT)

        # ---- qW[(b q), h'] = q @ W and qb[(b q)] = q . b ----
        qW = wpool.tile([128, H], ldt)
        for hf in range(2):
            ps_qw = pst.tile([128, 512], f32, tag="st")
            for k in range(HK):
                nc.tensor.matmul(ps_qw[:, :NH], qT[:, k, :],
                                 w_sb[:, k, hf * NH:(hf + 1) * NH],
                                 start=(k == 0), stop=(k == HK - 1))
            nc.scalar.copy(out=qW[:, hf * NH:(hf + 1) * NH], in_=ps_qw[:, :NH])
        ps_qb = pst.tile([64, BPC], f32, tag="st")
        for bi in range(BPC):
            for k in range(HK):
                # N=1 violates the fp32r moving-dim ISA restriction; these
                # 12 tiny matmuls run as plain fp32 via bitcast views.
                nc.tensor.matmul(ps_qb[:, bi:bi + 1],
                                 qT[:, k, bi * 64:(bi + 1) * 64].bitcast(f32),
                                 b_sb[:, k:k + 1],
                                 start=(k == 0), stop=(k == HK - 1))
        qrow = small.tile([64, BPC], f32, tag="qrow", bufs=1)
        nc.vector.tensor_add(qrow, ps_qb, qmbias)

        # ---- qWT[h', (b q)] via PE transposes of qW ----
        qWT = wpool.tile([128, HK, 128], ldt)
        for g in range(2):
            tp = ptp.tile([128, 3, 128], ldt, tag="tp")
            for j in range(3):
                k = g * 3 + j
                nc.tensor.transpose(tp[:, j, :],
                                    qW[:, k * 128:(k + 1) * 128], ident_r)
            nc.scalar.copy(out=qWT[:, g * 3:(g + 1) * 3, :], in_=tp)

        # ---- logits sT[q, c] (f32r, 512-wide) + c-mask bias, per batch ----
        ps_sts = []
        for bi in range(BPC):
            ps_st = pst.tile([64, CL], f32, tag="st")
            for k in range(HK):
                nc.tensor.matmul(ps_st, qWT[:, k, bi * 64:(bi + 1) * 64],
                                 cTs[bi][:, k, :], start=(k == 0), stop=False)
            nc.tensor.matmul(ps_st, ones[:1, :QL], cbias[:1, bi],
                             start=False, stop=True)
            ps_sts.append(ps_st)

        # out[:, 0:H] = c — no compute dependency, write early
        out_rs = []
        for bi in range(BPC):
            out_r = outD[bi].rearrange("(ct p) h4 -> p ct h4", p=128)
            out_rs.append(out_r)
            nc.sync.dma_start(out=out_r[:, :, 0:H], in_=c_fs[bi])

        # ---- softmaxes + value matmuls + products + output DMAs ----
        for bi in range(BPC):
            c_r, c_f, cT, ps_st = c_rs[bi], c_fs[bi], cTs[bi], ps_sts[bi]
            out_r = out_rs[bi]

            # column softmax s2 (over c = free axis), unnormalized, f32r
            nmax2 = small.tile([64, 1], f32, tag="nmax2")
            nc.vector.reduce_max(nmax2, ps_st, axis=mybir.AxisListType.X,
                                 negate=True)
            e2T = small.tile([64, CL], ldt, tag="e2T")
            sum2 = small.tile([64, 1], f32, tag="sum2")
            nc.scalar.activation(e2T, ps_st, mybir.ActivationFunctionType.Exp,
                                 bias=nmax2, scale=1.0, accum_out=sum2)
            r2 = small.tile([64, 1], f32, tag="r2")
            nc.vector.reciprocal(r2, sum2)
            # e2[c, q] chunks for the qc matmul (transpose back)
            tpe = ptp.tile([128, CT, QL], ldt, tag="tp")
            for ci in range(CT):
                nc.tensor.transpose(tpe[:, ci, :],
                                    e2T[:, ci * 128:(ci + 1) * 128],
                                    ident_r[:64, :64])
            e2s = small.tile([128, CT, QL], ldt, tag="e2s")
            nc.vector.tensor_copy(out=e2s, in_=tpe)

            # biased logits (+ per-partition q-side bias) for the s1 path
            sTb = small.tile([64, CL], f32, tag="sTb")
            nc.scalar.add(sTb, ps_st, qrow[:, bi:bi + 1])

            # row softmax s1 in the [c, q] layout, unnormalized (bf16);
            # the 1/sum1 factor rides the a copy-out and the c*bv product
            tps = ptp.tile([128, CT, QL], f32, tag="tp")
            for ci in range(CT):
                nc.tensor.transpose(tps[:, ci, :],
                                    sTb[:, ci * 128:(ci + 1) * 128],
                                    ident[:64, :64])
            e1 = small.tile([128, CT, QL], bf16, tag="e1")
            sum1 = small.tile([128, CT], f32, tag="sum1")
            for ci in range(CT):
                nmax1 = small.tile([128, 1], f32, tag="nmax1")
                nc.vector.reduce_max(nmax1, tps[:, ci, :],
                                     axis=mybir.AxisListType.X, negate=True)
                nc.scalar.activation(e1[:, ci, :], tps[:, ci, :],
                                     mybir.ActivationFunctionType.Exp,
                                     bias=nmax1, scale=1.0,
                                     accum_out=sum1[:, ci:ci + 1])
            r1 = small.tile([128, CT], f32, tag="r1")
            nc.vector.reciprocal(r1, sum1)
            tpt = ptp.tile([64, CL], bf16, tag="tp")
            for ci in range(CT):
                nc.tensor.transpose(tpt[:, ci * 128:(ci + 1) * 128],
                                    e1[:, ci, :], identb)
            s1T = small.tile([64, CL], bf16, tag="s1T")
            nc.vector.tensor_copy(out=s1T, in_=tpt)

            # qc[q, h] = s2.T @ c  (f32r, scale r2 on the bf16 copy-out)
            qc_bf = small.tile([64, H], bf16, tag="qc")
            for hf in range(2):
                ps_qc = pval.tile([64, 512], f32, tag="val")
                for ci in range(CT):
                    nc.tensor.matmul(ps_qc[:, :NH], e2s[:, ci, :],
                                     c_r[:, ci, hf * NH:(hf + 1) * NH],
                                     start=(ci == 0), stop=(ci == CT - 1))
                nc.scalar.mul(qc_bf[:, hf * NH:(hf + 1) * NH],
                              ps_qc[:, :NH], r2)

            # a = s1 @ q ; bv = s1 @ qc ; c*a ; c*bv  (r1 folded in)
            a_f32 = outp.tile([128, CT, H], f32, tag="af")
            ca_f32 = outp.tile([128, CT, H], f32, tag="caf")
            cbv_f32 = outp.tile([128, CT, H], f32, tag="cbvf")
            for ci in range(CT):
                r1c = r1[:, ci:ci + 1]
                for hf in range(2):
                    cols = slice(hf * NH, (hf + 1) * NH)
                    ps_a = pval.tile([128, 512], f32, tag="val")
                    nc.tensor.matmul(ps_a[:, :NH],
                                     s1T[:, ci * 128:(ci + 1) * 128],
                                     q_bf[:, bi, cols], start=True, stop=True)
                    if hf == 0:
                        nc.scalar.mul(a_f32[:, ci, cols], ps_a[:, :NH], r1c)
                    else:
                        nc.vector.tensor_scalar_mul(a_f32[:, ci, cols],
                                                    ps_a[:, :NH], r1c)
                    ps_bv = pval.tile([128, 512], f32, tag="val")
                    nc.tensor.matmul(ps_bv[:, :NH],
                                     s1T[:, ci * 128:(ci + 1) * 128],
                                     qc_bf[:, cols], start=True, stop=True)
                    nc.vector.scalar_tensor_tensor(
                        out=cbv_f32[:, ci, cols], in0=ps_bv[:, :NH],
                        scalar=r1c, in1=c_f[:, ci, cols],
                        op0=MULT, op1=MULT)
                nc.vector.tensor_mul(ca_f32[:, ci, :], c_f[:, ci, :],
                                     a_f32[:, ci, :])
            for half in range(2):
                cs = slice(2 * half, 2 * half + 2)
                nc.sync.dma_start(out=out_r[:, cs, H:2 * H],
                                  in_=a_f32[:, cs, :])
                nc.sync.dma_start(out=out_r[:, cs, 2 * H:3 * H],
                                  in_=ca_f32[:, cs, :])
                nc.sync.dma_start(out=out_r[:, cs, 3 * H:4 * H],
                                  in_=cbv_f32[:, cs, :])

    nc.finalize()
    return nc


_NC_CACHE: dict = {}


def _get_nc(precision: int = 1) -> bass.Bass:
    if precision not in _NC_CACHE:
        _NC_CACHE[precision] = _build_nc(precision)
    return _NC_CACHE[precision]


def kernel(c, q, c_mask, q_mask, W, b, _trace=False, _precision=1, _tmpdir=None):
    nc = _get_nc(_precision)
    in_maps = []
    for i in range(NCORES):
        sl = slice(i * BPC, (i + 1) * BPC)
        in_maps.append({
            "c": np.ascontiguousarray(np.asarray(c)[sl], dtype=np.float32),
            "q": np.ascontiguousarray(np.asarray(q)[sl], dtype=np.float32),
            "c_mask": np.ascontiguousarray(np.asarray(c_mask)[sl], dtype=np.int32),
            "q_mask": np.ascontiguousarray(np.asarray(q_mask)[sl], dtype=np.int32),
            "W": np.ascontiguousarray(np.asarray(W), dtype=np.float32),
            "b": np.ascontiguousarray(np.asarray(b), dtype=np.float32),
        })
    res = run_bass_kernel_spmd(nc, in_maps, core_ids=list(range(NCORES)),
                               trace=_trace, tmpdir=_tmpdir)
    out = np.concatenate([res.results[i]["out"] for i in range(NCORES)], axis=0)
    if _trace:
        return out, res
    return out


# revision 22
# speedup vs baseline: 1.5113x; 1.0661x over previous
"""BiDAF attention (nn_BertBidafAttention) on 8 TRN2 NeuronCores.

Math (per batch, reference):
    cp = c @ W.T + b            [CL, H]
    s  = cp @ q.T               [CL, QL]
    s1 = softmax_q(s + qmask_bias)      (row softmax)
    s2 = softmax_c(s + cmask_bias)      (col softmax)
    a  = s1 @ q                 [CL, H]
    bv = (s1 @ s2.T) @ c        [CL, H]
    x  = [c, a, c*a, c*bv]      [CL, 4H]

Restructured for the PE:
    qW[(b q), h'] = sum_h q[(b q), h] W[h, h']    (both batches fused, 75 MF)
    sT[q, c]      = sum_h qWT[h, q] cT[h, c] + cmask_bias[c]  (f32r, 512-wide)
    bv            = s1 @ (s2.T @ c)               (avoids the [CL,CL] product)
The c-mask bias is constant along q for fixed c so it cancels in s1's
softmax; the q-side bias (q.b + qmask_bias) is added per-partition into
sTb and cancels in s2's softmax.  NEGB=-1000 makes exp(masked-max)==0.

Precision split (validated vs the fp32 reference in numpy):
    logit matmuls (qW, sT) and the qc matmul run f32r (TF32-ish rounding);
    the a/bv matmuls run bf16 on the softmax weights.  (bf16 logits would
    be 7e-2 rel err and fail the 2e-2 gate.)
Both softmaxes stay unnormalized: e2's 1/sum rides the qc copy-out, e1's
1/sum rides the a copy-out and the c*bv product (scalar_tensor_tensor),
so no normalization passes exist at all.

Scheduling notes (from per-instruction NTFF traces):
  - every HWDGE dma_start costs ~0.8us of *sequencer* time, so DMAs are
    batched (1 per input tensor, per-ci-pair for outputs) and all live on
    the sync ring (no compute), ordered by expected readiness; tiny or
    casting loads (b, masks, q_bf) go through SWDGE on gpsimd.
  - GpSimd tensor ops are ~4x slower than DVE -> gpsimd only does SWDGE.
  - c is held in SBUF as f32r (same bits as fp32): the qc matmul streams
    it directly, fp32 consumers use bitcast views, so no bf16 copy of c.
  - ps_st has its own PSUM tag so batch 1's logit matmuls don't wait on
    batch 0's softmax to release the bank.

Sharding: data-parallel over batch, 2 batches per core, no collectives.
"""

import numpy as np
from contextlib import ExitStack

import concourse.bass as bass
from concourse import bacc
import concourse.mybir as mybir
import concourse.tile as tile
from concourse.masks import make_identity
from concourse.bass_utils import run_bass_kernel_spmd

B, CL, QL, H = 16, 512, 64, 768
NCORES = 8
BPC = B // NCORES  # batches per core
HK = H // 128      # 6 k-tiles over the feature dims
CT = CL // 128     # 4 c-tiles
NH = H // 2        # 384, N per matmul half (one PSUM bank)
NEGB = -1000.0     # additive mask bias; exp(NEGB - max) == 0.0 in fp32

f32 = mybir.dt.float32
f32r = mybir.dt.float32r
bf16 = mybir.dt.bfloat16
i32 = mybir.dt.int32
MULT = mybir.AluOpType.mult


def _build_nc(precision: int = 1) -> bass.Bass:
    nc = bacc.Bacc()
    cD = nc.declare_dram_parameter("c", [BPC, CL, H], f32, isOutput=False)
    qD = nc.declare_dram_parameter("q", [BPC, QL, H], f32, isOutput=False)
    cmD = nc.declare_dram_parameter("c_mask", [BPC, CL], i32, isOutput=False)
    qmD = nc.declare_dram_parameter("q_mask", [BPC, QL], i32, isOutput=False)
    WD = nc.declare_dram_parameter("W", [H, H], f32, isOutput=False)
    bD = nc.declare_dram_parameter("b", [H], f32, isOutput=False)
    outD = nc.declare_dram_parameter("out", [BPC, CL, 4 * H], f32, isOutput=True)

    ldt = f32r if precision >= 1 else f32  # logit-path matmul dtype

    with tile.TileContext(nc) as tc, ExitStack() as ctx:
        const = ctx.enter_context(tc.tile_pool(name="const", bufs=1))
        wpool = ctx.enter_context(tc.tile_pool(name="wpool", bufs=1))
        cpool = ctx.enter_context(tc.tile_pool(name="cpool", bufs=1))
        small = ctx.enter_context(tc.tile_pool(name="small", bufs=2))
        outp = ctx.enter_context(tc.tile_pool(name="outp", bufs=2))
        ptp = ctx.enter_context(tc.tile_pool(name="ptp", bufs=3, space="PSUM"))
        pst = ctx.enter_context(tc.tile_pool(name="pst", bufs=2, space="PSUM"))
        pval = ctx.enter_context(tc.tile_pool(name="pval", bufs=3, space="PSUM"))

        ident = const.tile([128, 128], f32)
        make_identity(nc, ident)
        identb = const.tile([128, 128], bf16)
        nc.vector.tensor_copy(identb, ident)
        ones_f = const.tile([1, 128], f32)
        nc.vector.memset(ones_f, 1.0)
        if ldt == f32:
            ident_r = ident
            ones = ones_f
        else:
            ident_r = const.tile([128, 128], ldt)
            nc.vector.tensor_copy(ident_r, ident)
            ones = const.tile([1, 128], ldt)
            nc.vector.tensor_copy(ones, ones_f)

        # ---- input DMAs: one per big tensor, all on the sync ring ----
        q_both = wpool.tile([128, H], ldt)  # [(b q), h]
        nc.sync.dma_start(out=q_both,
                          in_=qD[:].rearrange("b q h -> (b q) h").bitcast(ldt))
        c_rs, c_fs = [], []
        for bi in range(BPC):
            c_r = cpool.tile([128, CT, H], ldt, name=f"c{bi}")
            c_rs.append(c_r)
            c_fs.append(c_r.bitcast(f32))
        w_sb = wpool.tile([128, HK, H], ldt)  # [p, k, h']  (W[k*128+p, h'])
        nc.sync.dma_start(out=c_rs[0],
                          in_=cD[0].rearrange("(ct p) h -> p ct h", p=128)
                          .bitcast(ldt))
        nc.sync.dma_start(out=w_sb,
                          in_=WD[:].rearrange("(k p) h -> p k h", p=128)
                          .bitcast(ldt))
        nc.sync.dma_start(out=c_rs[1],
                          in_=cD[1].rearrange("(ct p) h -> p ct h", p=128)
                          .bitcast(ldt))

        # small/irregular loads via SWDGE (gpsimd), casting where needed
        b_sb = wpool.tile([128, HK], f32)
        nc.gpsimd.dma_start(out=b_sb,
                            in_=bD[:].rearrange("(k p) -> p k", p=128))
        q_bf = wpool.tile([64, BPC, H], bf16)
        for bi in range(BPC):
            nc.gpsimd.dma_start(out=q_bf[:, bi, :], in_=qD[bi])
        qmf = small.tile([64, BPC], f32, tag="qmf", bufs=1)
        nc.gpsimd.dma_start(out=qmf, in_=qmD[:].rearrange("b q -> q b"))
        cmf = small.tile([1, BPC, CL], f32, tag="cmf", bufs=1)
        nc.gpsimd.dma_start(out=cmf[:1].rearrange("o b l -> o (b l)"),
                            in_=cmD[:].rearrange("b (o l) -> o (b l)", o=1))
        # bias = (mask - 1) * |NEGB|  ->  0 where mask==1, NEGB where mask==0
        qmbias = small.tile([64, BPC], f32, tag="qmbias", bufs=1)
        nc.scalar.activation(qmbias, qmf, mybir.ActivationFunctionType.Copy,
                             bias=NEGB, scale=-NEGB)
        cbias = small.tile([1, BPC, CL], ldt, tag="cbias", bufs=1)
        nc.scalar.activation(cbias, cmf, mybir.ActivationFunctionType.Copy,
                             bias=NEGB, scale=-NEGB)

        # ---- qT[h, (b q)] via PE transposes of q_both ----
        qT = wpool.tile([128, HK, 128], ldt)
        for g in range(2):  # two groups of 3 k-chunks -> one PSUM bank each
            tp = ptp.tile([128, 3, 128], ldt, tag="tp")
            for j in range(3):
                k = g * 3 + j
                nc.tensor.transpose(tp[:, j, :],
                                    q_both[:, k * 128:(k + 1) * 128], ident_r)
            nc.vector.tensor_copy(out=qT[:, g * 3:(g + 1) * 3, :], in_=tp)

        # ---- cT[h, c] via PE transposes, grouped per c-chunk ----
        cTs = []
        for bi in range(BPC):
            c_r = c_rs[bi]
            cT = cpool.tile([128, HK, CL], ldt, name=f"ct{bi}")
            for ci in range(CT):
                for g in range(2):
                    tp = ptp.tile([128, 3, 128], ldt, tag="tp")
                    for j in range(3):
                        k = g * 3 + j
                        nc.tensor.transpose(
                            tp[:, j, :],
                            c_r[:, ci, k * 128:(k + 1) * 128], ident_r)
                    dst = cT[:, g * 3:(g + 1) * 3, ci * 128:(ci + 1) * 128]
                    if (ci + g) % 2 == 0:
                        nc.vector.tensor_copy(out=dst, in_=tp)
                    else:
                        nc.scalar.copy(out=dst, in_=tp)
            cTs.append(cT)

        # ---- qW[(b q), h'] = q @ W and qb[(b q)] = q . b ----
        qW = wpool.tile([128, H], ldt)
        for hf in range(2):
            ps_qw = pst.tile([128, 512], f32, tag="st")
            for k in range(HK):
                nc.tensor.matmul(ps_qw[:, :NH], qT[:, k, :],
                                 w_sb[:, k, hf * NH:(hf + 1) * NH],
                                 start=(k == 0), stop=(k == HK - 1))
            nc.scalar.copy(out=qW[:, hf * NH:(hf + 1) * NH], in_=ps_qw[:, :NH])
        ps_qb = pst.tile([64, BPC], f32, tag="st")
        for bi in range(BPC):
            for k in range(HK):
                # N=1 violates the fp32r moving-dim ISA restriction; these
                # 12 tiny matmuls run as plain fp32 via bitcast views.
                nc.tensor.matmul(ps_qb[:, bi:bi + 1],
                                 qT[:, k, bi * 64:(bi + 1) * 64].bitcast(f32),
                                 b_sb[:, k:k + 1],
                                 start=(k == 0), stop=(k == HK - 1))
        qrow = small.tile([64, BPC], f32, tag="qrow", bufs=1)
        nc.vector.tensor_add(qrow, ps_qb, qmbias)

        # ---- qWT[h', (b q)] via PE transposes of qW ----
        qWT = wpool.tile([128, HK, 128], ldt)
        for g in range(2):
            tp = ptp.tile([128, 3, 128], ldt, tag="tp")
            for j in range(3):
                k = g * 3 + j
                nc.tensor.transpose(tp[:, j, :],
                                    qW[:, k * 128:(k + 1) * 128], ident_r)
            nc.scalar.copy(out=qWT[:, g * 3:(g + 1) * 3, :], in_=tp)

        # ---- logits sT[q, c] (f32r, 512-wide) + c-mask bias, per batch ----
        ps_sts = []
        for bi in range(BPC):
            ps_st = pst.tile([64, CL], f32, tag="st")
            for k in range(HK):
                nc.tensor.matmul(ps_st, qWT[:, k, bi * 64:(bi + 1) * 64],
                                 cTs[bi][:, k, :], start=(k == 0), stop=False)
            nc.tensor.matmul(ps_st, ones[:1, :QL], cbias[:1, bi],
                             start=False, stop=True)
            ps_sts.append(ps_st)

        # out[:, 0:H] = c — no compute dependency, write early
        out_rs = []
        for bi in range(BPC):
            out_r = outD[bi].rearrange("(ct p) h4 -> p ct h4", p=128)
            out_rs.append(out_r)
            nc.sync.dma_start(out=out_r[:, :, 0:H], in_=c_fs[bi])

        # ---- softmaxes + value matmuls + products + output DMAs ----
        # Per batch: the s1 path streams per c-chunk — each 128-row chunk's
        # softmax, a-matmul, c*a product and their output DMAs fire as soon
        # as that chunk is ready.  Only the c*bv quarter waits for the full
        # s2/qc chain.
        cbv_dmas = []
        for bi in range(BPC):
            c_r, c_f, cT, ps_st = c_rs[bi], c_fs[bi], cTs[bi], ps_sts[bi]
            out_r = out_rs[bi]

            # column softmax s2 (over c = free axis), unnormalized, f32r
            nmax2 = small.tile([64, 1], f32, tag="nmax2")
            nc.vector.reduce_max(nmax2, ps_st, axis=mybir.AxisListType.X,
                                 negate=True)
            e2T = small.tile([64, CL], ldt, tag="e2T")
            sum2 = small.tile([64, 1], f32, tag="sum2")
            nc.scalar.activation(e2T, ps_st, mybir.ActivationFunctionType.Exp,
                                 bias=nmax2, scale=1.0, accum_out=sum2)
            r2 = small.tile([64, 1], f32, tag="r2")
            nc.vector.reciprocal(r2, sum2)

            # biased logits (+ per-partition q-side bias) for the s1 path
            sTb = small.tile([64, CL], f32, tag="sTb")
            nc.scalar.add(sTb, ps_st, qrow[:, bi:bi + 1])

            # s1 per c-chunk: transpose -> max -> exp -> transpose back ->
            # a matmuls -> a (r1-scaled) -> c*a -> DMAs.  bf16, unnormalized.
            a_f32 = outp.tile([128, CT, H], f32, tag="af")
            ca_f32 = outp.tile([128, CT, H], f32, tag="caf")
            s1T = small.tile([64, CL], bf16, tag="s1T")
            r1 = small.tile([128, CT], f32, tag="r1")
            for ci in range(CT):
                cisl = slice(ci * 128, (ci + 1) * 128)
                tps = ptp.tile([128, QL], f32, tag="tp")
                nc.tensor.transpose(tps, sTb[:, cisl], ident[:64, :64])
                nmax1 = small.tile([128, 1], f32, tag="nmax1")
                nc.vector.reduce_max(nmax1, tps, axis=mybir.AxisListType.X,
                                     negate=True)
                e1 = small.tile([128, QL], bf16, tag="e1")
                sum1 = small.tile([128, 1], f32, tag="sum1")
                nc.scalar.activation(e1, tps,
                                     mybir.ActivationFunctionType.Exp,
                                     bias=nmax1, scale=1.0, accum_out=sum1)
                nc.vector.reciprocal(r1[:, ci:ci + 1], sum1)
                tpt = ptp.tile([64, 128], bf16, tag="tp")
                nc.tensor.transpose(tpt, e1, identb)
                nc.vector.tensor_copy(out=s1T[:, cisl], in_=tpt)
                r1c = r1[:, ci:ci + 1]
                for hf in range(2):
                    cols = slice(hf * NH, (hf + 1) * NH)
                    ps_a = pval.tile([128, 512], f32, tag="val")
                    nc.tensor.matmul(ps_a[:, :NH], s1T[:, cisl],
                                     q_bf[:, bi, cols], start=True, stop=True)
                    if hf == 0:
                        nc.scalar.mul(a_f32[:, ci, cols], ps_a[:, :NH], r1c)
                    else:
                        nc.vector.tensor_scalar_mul(a_f32[:, ci, cols],
                                                    ps_a[:, :NH], r1c)
                nc.gpsimd.tensor_mul(ca_f32[:, ci, :], c_f[:, ci, :],
                                     a_f32[:, ci, :])
                nc.sync.dma_start(out=out_r[:, ci:ci + 1, H:2 * H],
                                  in_=a_f32[:, ci:ci + 1, :])
                nc.sync.dma_start(out=out_r[:, ci:ci + 1, 2 * H:3 * H],
                                  in_=ca_f32[:, ci:ci + 1, :])

            # e2[c, q] chunks for the qc matmul (transpose back)
            tpe = ptp.tile([128, CT, QL], ldt, tag="tp")
            for ci in range(CT):
                nc.tensor.transpose(tpe[:, ci, :],
                                    e2T[:, ci * 128:(ci + 1) * 128],
                                    ident_r[:64, :64])
            e2s = small.tile([128, CT, QL], ldt, tag="e2s")
            nc.vector.tensor_copy(out=e2s, in_=tpe)

            # qc[q, h] = s2.T @ c  (f32r, scale r2 on the bf16 copy-out)
            qc_bf = small.tile([64, H], bf16, tag="qc")
            for hf in range(2):
                ps_qc = pval.tile([64, 512], f32, tag="val")
                for ci in range(CT):
                    nc.tensor.matmul(ps_qc[:, :NH], e2s[:, ci, :],
                                     c_r[:, ci, hf * NH:(hf + 1) * NH],
                                     start=(ci == 0), stop=(ci == CT - 1))
                nc.scalar.mul(qc_bf[:, hf * NH:(hf + 1) * NH],
                              ps_qc[:, :NH], r2)

            # bv = s1 @ qc ; c*bv  (r1 folded into the product)
            cbv_f32 = outp.tile([128, CT, H], f32, tag="cbvf")
            for ci in range(CT):
                r1c = r1[:, ci:ci + 1]
                for hf in range(2):
                    cols = slice(hf * NH, (hf + 1) * NH)
                    ps_bv = pval.tile([128, 512], f32, tag="val")
                    nc.tensor.matmul(ps_bv[:, :NH],
                                     s1T[:, ci * 128:(ci + 1) * 128],
                                     qc_bf[:, cols], start=True, stop=True)
                    nc.vector.scalar_tensor_tensor(
                        out=cbv_f32[:, ci, cols], in0=ps_bv[:, :NH],
                        scalar=r1c, in1=c_f[:, ci, cols],
                        op0=MULT, op1=MULT)
            cbv_dmas.append((out_r, cbv_f32))
        # c*bv quarters are the last tensors ready — their DMAs go last
        for out_r, cbv_f32 in cbv_dmas:
            for half in range(2):
                cs = slice(2 * half, 2 * half + 2)
                nc.sync.dma_start(out=out_r[:, cs, 3 * H:4 * H],
                                  in_=cbv_f32[:, cs, :])

    nc.finalize()
    return nc


_NC_CACHE: dict = {}


def _get_nc(precision: int = 1) -> bass.Bass:
    if precision not in _NC_CACHE:
        _NC_CACHE[precision] = _build_nc(precision)
    return _NC_CACHE[precision]


def kernel(c, q, c_mask, q_mask, W, b, _trace=False, _precision=1, _tmpdir=None):
    nc = _get_nc(_precision)
    in_maps = []
    for i in range(NCORES):
        sl = slice(i * BPC, (i + 1) * BPC)
        in_maps.append({
            "c": np.ascontiguousarray(np.asarray(c)[sl], dtype=np.float32),
            "q": np.ascontiguousarray(np.asarray(q)[sl], dtype=np.float32),
            "c_mask": np.ascontiguousarray(np.asarray(c_mask)[sl], dtype=np.int32),
            "q_mask": np.ascontiguousarray(np.asarray(q_mask)[sl], dtype=np.int32),
            "W": np.ascontiguousarray(np.asarray(W), dtype=np.float32),
            "b": np.ascontiguousarray(np.asarray(b), dtype=np.float32),
        })
    res = run_bass_kernel_spmd(nc, in_maps, core_ids=list(range(NCORES)),
                               trace=_trace, tmpdir=_tmpdir)
    out = np.concatenate([res.results[i]["out"] for i in range(NCORES)], axis=0)
    if _trace:
        return out, res
    return out
